# revision 17
# baseline (speedup 1.0000x reference)
"""Bass/Trainium2 kernel for nn_FBRNN: 6-layer feedback GRU, T=16384, H=16.

Structure:
  - Phase A (input projection, memory-bound): gi0 = batch @ W_ih0.T + b_ih0,
    computed with jax on a neuron core (XLA handles the big matmul), falling
    back to numpy.
  - Phase B (serial recurrence): a Bass kernel on one NeuronCore runs the
    16384 sequential steps with ~75 instructions/step, chunk-unrolled inside
    a hardware For_i loop. All gate algebra is folded into PE matmuls with
    pre-signed weights; sigmoid/tanh on ACT (one table set); softmax uses the
    exact identity exp(x) = (1+tanh(x/2))/(1-tanh(x/2)) to avoid an
    ACT-table switch.
  - Phase C: out[t] = h5[t] @ w_eff + b_eff on host (tiny).
"""
import sys
sys.path.insert(0, "/opt/trn_rl_repo")

import numpy as np

T_FULL, F_IN, H, A_DIM, L = 16384, 2048, 16, 64, 6
NEG = -30.0  # mask constant; tanh(-15) == -1.0 in fp32

_CACHE = {}


def _build_consts(h0, W_ih0, W_hh0, b_ih0, b_hh0, W_ih, W_hh, b_ih, b_hh, Wa, ba, va):
    f32 = np.float32

    def lhsT_gate(Wih, bih, Whh, bhh, gr, sign):
        M = np.zeros((49, 16), f32)
        if Wih is not None:
            M[0:16, :] = sign * Wih[gr:gr + 16, :].T
            M[16, :] = sign * bih[gr:gr + 16]
        M[32:48, :] = sign * Whh[gr:gr + 16, :].T
        M[48, :] = sign * bhh[gr:gr + 16]
        return M

    # WB: (49, 16*23): l0 [rb, zb, sb(=gh0)] then l=1..5 [r, z, s, gh]
    blocks = [
        lhsT_gate(None, None, W_hh0, b_hh0, 0, -1.0),
        lhsT_gate(None, None, W_hh0, b_hh0, 16, -1.0),
        lhsT_gate(None, None, W_hh0, b_hh0, 32, +1.0),
    ]
    for l in range(1, L):
        Wih, Whh = W_ih[l - 1], W_hh[l - 1]
        bih, bhh = b_ih[l - 1], b_hh[l - 1]
        blocks.append(lhsT_gate(Wih, bih, Whh, bhh, 0, -1.0))
        blocks.append(lhsT_gate(Wih, bih, Whh, bhh, 16, -1.0))
        blocks.append(lhsT_gate(Wih, bih, Whh, bhh, 32, +1.0))
        gh = np.zeros((49, 16), f32)
        gh[32:48, :] = Whh[32:48, :].T
        gh[48, :] = bhh[32:48]
        blocks.append(gh)
    WB = np.concatenate(blocks, axis=1)  # (49, 368)

    # WS: layer-0 selectors over gi0 (48-dim): r(-), z(-), s(+)
    WS = np.zeros((48, 48), f32)
    for gi, (gr, sign) in enumerate([(0, -1.0), (16, -1.0), (32, 1.0)]):
        for m in range(16):
            WS[gr + m, 16 * gi + m] = sign

    # WaAug (17, 320): i = 0..4 blocks; row 16 = ba
    WaAug = np.zeros((17, 320), f32)
    for i in range(5):
        WaAug[0:16, 64 * i:64 * (i + 1)] = Wa[i]
        WaAug[16, 64 * i:64 * (i + 1)] = ba[i]

    # VAb (6, 320): va * 0.5 broadcast over k rows
    VAb = np.zeros((6, 320), f32)
    for i in range(5):
        VAb[:, 64 * i:64 * (i + 1)] = 0.5 * va[i][None, :]

    # Mhalf (6, 6): (mask/2); col 5 drives h_next[5] = newS[5] exactly
    Mh = np.zeros((6, 6), f32)
    for i in range(5):
        for k in range(6):
            if k < i:
                Mh[k, i] = NEG
    Mh[:, 5] = NEG
    Mh[5, 5] = 1.0

    r2init = np.zeros((49, 7), f32)
    r2init[16, :] = 1.0
    r2init[48, :] = 1.0
    for l in range(L):
        r2init[32:48, l] = h0[l, 0, :]

    I16 = np.eye(16, dtype=f32)
    I6 = np.eye(6, dtype=f32)
    import ml_dtypes
    return dict(wb=WB, ws=WS, wa=WaAug.astype(ml_dtypes.bfloat16),
                vab=VAb.astype(ml_dtypes.bfloat16), mh=Mh, r2init=r2init,
                i16=I16, i6=I6)


def _build_bass(T, U):
    import concourse.bacc as bacc
    import concourse.mybir as mybir
    import concourse.tile as tile
    import concourse.bass as bass

    F32 = mybir.dt.float32
    BF16 = mybir.dt.bfloat16
    AF = mybir.ActivationFunctionType
    ALU = mybir.AluOpType

    nc = bacc.Bacc()
    gi0_d = nc.declare_dram_parameter("gi0t", [48, T], F32, isOutput=False)
    wb_d = nc.declare_dram_parameter("wb", [49, 368], F32, isOutput=False)
    ws_d = nc.declare_dram_parameter("ws", [48, 48], F32, isOutput=False)
    wa_d = nc.declare_dram_parameter("wa", [17, 320], BF16, isOutput=False)
    vab_d = nc.declare_dram_parameter("vab", [6, 320], BF16, isOutput=False)
    mh_d = nc.declare_dram_parameter("mh", [6, 6], F32, isOutput=False)
    r2i_d = nc.declare_dram_parameter("r2init", [49, 7], F32, isOutput=False)
    i16_d = nc.declare_dram_parameter("i16", [16, 16], F32, isOutput=False)
    i6_d = nc.declare_dram_parameter("i6", [6, 6], F32, isOutput=False)
    h5_d = nc.declare_dram_parameter("h5all", [16, T], F32, isOutput=True)

    with tile.TileContext(nc) as tc:
        with (
            tc.tile_pool(name="const", bufs=1) as cp,
            tc.tile_pool(name="state", bufs=1) as st,
            tc.tile_pool(name="work", bufs=2) as wk,
            tc.tile_pool(name="ps", bufs=1, space="PSUM") as ps,
        ):
            wb = cp.tile([49, 368], F32)
            ws = cp.tile([48, 48], F32)
            wa = cp.tile([17, 320], BF16)
            vab = cp.tile([6, 320], BF16)
            ones6 = cp.tile([6, 1], F32)
            mh = cp.tile([6, 6], F32)
            i16 = cp.tile([16, 16], F32)
            i6 = cp.tile([6, 6], F32)
            nc.sync.dma_start(wb[:], wb_d[:])
            nc.sync.dma_start(ws[:], ws_d[:])
            nc.sync.dma_start(wa[:], wa_d[:])
            nc.sync.dma_start(vab[:], vab_d[:])
            nc.sync.dma_start(mh[:], mh_d[:])
            nc.sync.dma_start(i16[:], i16_d[:])
            nc.sync.dma_start(i6[:], i6_d[:])
            nc.vector.memset(ones6[:], 1.0)

            R2 = st.tile([49, 7], F32)
            nc.sync.dma_start(R2[:], r2i_d[:])
            Hprev = st.tile([16, 6], F32)
            nc.vector.tensor_copy(Hprev[:], R2[32:48, 0:6])
            e_t = st.tile([6, 6], F32)
            nc.vector.memset(e_t[:], 0.0)
            Snew = st.tile([6, 17], F32)
            nc.vector.memset(Snew[:], 1.0)
            Sbf = st.tile([17, 6], BF16)
            nc.vector.memset(Sbf[:], 1.0)

            # PSUM: separate tiles -> separate banks (avoid bank-overlap serialization)
            PS_G = ps.tile([16, 18], F32)    # gate cols 3l..3l+2
            PS_GH = ps.tile([16, 6], F32)    # gh per layer
            PS_A = ps.tile([6, 320], F32)    # attention pre-tanh
            PS_S = ps.tile([6, 16], F32)     # newS transposed
            PS_HU = ps.tile([6, 34], F32)    # h_u15 0-15 | h_u0 16-31 | Z0 32 | Z15 33
            PS_T0 = ps.tile([16, 1], F32)    # h_next[0] column
            PS_T15 = ps.tile([16, 5], F32)   # h_next[1..5] columns

            def lw(b):  # lhsT block b of WB
                return wb[:, 16 * b:16 * (b + 1)]

            negGH = st.tile([16, 6], F32)

            def emit_B15(pend):
                """Deferred h_next[1..5] state update; emitted between the
                next step's layer 0 and layer 1 so it overlaps."""
                MM = nc.tensor.matmul
                wt, iZ15 = pend
                MM(PS_HU[0:5, 0:16], wt[:, 1:6], Snew[:, 0:16], start=True, stop=True)
                hN = wk.tile([5, 16], F32, tag="hN")
                nc.vector.tensor_scalar(hN[:], PS_HU[0:5, 0:16], iZ15[:], None,
                                        op0=ALU.mult)
                nc.tensor.transpose(PS_T15[0:16, 0:5], hN[:], i6[0:5, 0:5])
                nc.vector.tensor_copy(R2[32:48, 1:6], PS_T15[0:16, 0:5])
                nc.vector.tensor_copy(Hprev[:, 1:6], PS_T15[0:16, 0:5])

            def emit_layer(l, k, gstage, pend):
                MM = nc.tensor.matmul
                c = 3 * l
                if l == 0:
                    MM(PS_GH[0:16, 0:1], lw(2), R2[:, 0:1], start=True, stop=True)
                    nc.vector.tensor_scalar_mul(negGH[:, 0:1], PS_GH[0:16, 0:1], -1.0)
                    MM(PS_G[0:16, 0:1], ws[:, 0:16], gstage[:, k:k + 1],
                       start=True, stop=False, skip_group_check=True)
                    MM(PS_G[0:16, 0:1], lw(0), R2[:, 0:1],
                       start=False, stop=True, skip_group_check=True)
                    MM(PS_G[0:16, 1:2], ws[:, 16:32], gstage[:, k:k + 1],
                       start=True, stop=False, skip_group_check=True)
                    MM(PS_G[0:16, 1:2], lw(1), R2[:, 0:1],
                       start=False, stop=True, skip_group_check=True)
                    MM(PS_G[0:16, 2:3], ws[:, 32:48], gstage[:, k:k + 1],
                       start=True, stop=False, skip_group_check=True)
                    MM(PS_G[0:16, 2:3], lw(2), R2[:, 0:1],
                       start=False, stop=True, skip_group_check=True)
                else:
                    if l == 1:
                        if pend is not None:
                            emit_B15(pend)
                        for j in range(1, L):
                            MM(PS_GH[0:16, j:j + 1], lw(3 + 4 * (j - 1) + 3),
                               R2[:, j:j + 1], start=True, stop=True)
                        nc.vector.tensor_scalar_mul(negGH[:, 1:6], PS_GH[0:16, 1:6], -1.0)
                    b = 3 + 4 * (l - 1)
                    MM(PS_G[0:16, c:c + 1], lw(b), R2[:, l:l + 1],
                       start=True, stop=True)
                    MM(PS_G[0:16, c + 1:c + 2], lw(b + 1), R2[:, l:l + 1],
                       start=True, stop=True)
                    MM(PS_G[0:16, c + 2:c + 3], lw(b + 2), R2[:, l:l + 1],
                       start=True, stop=True)
                RZ = wk.tile([16, 2], F32, tag="RZ")
                nc.scalar.activation(RZ[:], PS_G[0:16, c:c + 2], AF.Sigmoid)
                s_sb = wk.tile([16, 1], F32, tag="s_sb")
                nc.vector.tensor_copy(s_sb[:], PS_G[0:16, c + 2:c + 3])
                n_sb = wk.tile([16, 1], F32, tag="n_sb")
                nc.scalar.activation(n_sb[:], negGH[:, l:l + 1], AF.Tanh,
                                     bias=s_sb[:], scale=RZ[:, 0:1])
                v_sb = wk.tile([16, 1], F32, tag="v_sb")
                nc.vector.scalar_tensor_tensor(
                    v_sb[:], Hprev[:, l:l + 1], RZ[:, 1:2], Hprev[:, l:l + 1],
                    op0=ALU.mult, op1=ALU.subtract)
                nc.vector.scalar_tensor_tensor(
                    R2[0:16, l + 1:l + 2], n_sb[:], RZ[:, 1:2], v_sb[:],
                    op0=ALU.mult, op1=ALU.subtract)
                nc.vector.tensor_copy(Sbf[0:16, l:l + 1], R2[0:16, l + 1:l + 2])

            def emit_attn_B0(k, h5s):
                MM = nc.tensor.matmul
                MM(PS_A[0:6, :], Sbf[:], wa[:], start=True, stop=True)
                TA = wk.tile([6, 320], BF16, tag="TA")
                nc.scalar.activation(TA[:, 0:128], PS_A[0:6, 0:128], AF.Tanh)
                nc.scalar.activation(TA[:, 128:320], PS_A[0:6, 128:320], AF.Tanh)
                scr = wk.tile([6, 64], BF16, tag="scr")
                for i in range(5):
                    nc.vector.scalar_tensor_tensor(
                        scr[:], TA[:, 64 * i:64 * (i + 1)], 1.0,
                        vab[:, 64 * i:64 * (i + 1)],
                        op0=ALU.mult, op1=ALU.mult, accum_out=e_t[:, i:i + 1])
                eM = wk.tile([6, 6], F32, tag="eM")
                nc.vector.tensor_add(eM[:], e_t[:], mh[:])
                t_sb = wk.tile([6, 6], F32, tag="t_sb")
                nc.scalar.activation(t_sb[:], eM[:], AF.Tanh)
                den = wk.tile([6, 6], F32, tag="den")
                nc.vector.tensor_scalar(den[:], t_sb[:], -1.0, 1.0,
                                        op0=ALU.mult, op1=ALU.add)
                rden = wk.tile([6, 6], F32, tag="rden")
                nc.vector.reciprocal(rden[:], den[:])
                wt = wk.tile([6, 6], F32, tag="wt")
                nc.vector.scalar_tensor_tensor(wt[:], t_sb[:], 1.0, rden[:],
                                               op0=ALU.add, op1=ALU.mult)
                nc.tensor.transpose(PS_S[0:6, 0:16], R2[0:16, 1:7], i16[:])
                nc.vector.tensor_copy(Snew[:, 0:16], PS_S[0:6, 0:16])
                nc.gpsimd.tensor_copy(h5s[:, k:k + 1], R2[0:16, 6:7])
                # fast path: finish h_next[0] first so the next step's layer 0
                # overlaps the remaining attention tail
                MM(PS_HU[0:1, 32:33], wt[:, 0:1], ones6[:], start=True, stop=True)
                MM(PS_HU[0:5, 33:34], wt[:, 1:6], ones6[:], start=True, stop=True)
                iZ0 = wk.tile([1, 1], F32, tag="iZ0")
                nc.vector.reciprocal(iZ0[:], PS_HU[0:1, 32:33])
                iZ15 = wk.tile([5, 1], F32, tag="iZ15")
                nc.vector.reciprocal(iZ15[:], PS_HU[0:5, 33:34])
                MM(PS_HU[0:1, 16:32], wt[:, 0:1], Snew[:, 0:16], start=True, stop=True)
                hN0 = wk.tile([1, 16], F32, tag="hN0")
                nc.vector.tensor_scalar(hN0[:], PS_HU[0:1, 16:32], iZ0[:], None,
                                        op0=ALU.mult)
                nc.tensor.transpose(PS_T0[0:16, 0:1], hN0[:], i6[0:1, 0:1])
                nc.vector.tensor_copy(R2[32:48, 0:1], PS_T0[0:16, 0:1])
                nc.vector.tensor_copy(Hprev[:, 0:1], PS_T0[0:16, 0:1])
                return (wt, iZ15)

            def emit_chunk(gstage, h5s):
                pend = None
                for k in range(U):
                    emit_layer(0, k, gstage, None)
                    for l in range(1, L):
                        emit_layer(l, k, gstage, pend if l == 1 else None)
                    pend = emit_attn_B0(k, h5s)
                emit_B15(pend)

            if U >= T:
                for c0 in range(0, T, U):
                    gstage = wk.tile([48, U], F32, tag="gstage")
                    nc.sync.dma_start(gstage[:], gi0_d[:, c0:c0 + U])
                    h5s = wk.tile([16, U], F32, tag="h5s")
                    emit_chunk(gstage, h5s)
                    nc.sync.dma_start(h5_d[:, c0:c0 + U], h5s[:])
            else:
                with tc.For_i(0, T, U) as iv:
                    gstage = wk.tile([48, U], F32, tag="gstage")
                    nc.sync.dma_start(gstage[:], gi0_d[:, bass.ds(iv, U)])
                    h5s = wk.tile([16, U], F32, tag="h5s")
                    emit_chunk(gstage, h5s)
                    nc.sync.dma_start(h5_d[:, bass.ds(iv, U)], h5s[:])

    nc.compile()
    return nc


def _run_phase_b(gi0t, consts, T, U):
    from concourse.bass_utils import run_bass_kernel_spmd
    key = (T, U)
    if key not in _CACHE:
        _CACHE[key] = _build_bass(T, U)
    nc = _CACHE[key]
    in_map = dict(gi0t=gi0t, **consts)
    res = run_bass_kernel_spmd(nc, [in_map], [0]).results[0]
    return res["h5all"]


_PHASE_A_JIT = []


def _phase_a(batch, W_ih0, b_ih0):
    x2d = np.ascontiguousarray(batch[:, 0, :])
    try:
        import jax
        if not _PHASE_A_JIT:
            _PHASE_A_JIT.append(jax.jit(lambda x, w, b: (x @ w.T + b).T))
        gi0t = np.asarray(jax.block_until_ready(_PHASE_A_JIT[0](x2d, W_ih0, b_ih0)))
    except Exception:
        gi0t = (x2d @ W_ih0.T + b_ih0).T
    return np.ascontiguousarray(gi0t.astype(np.float32))  # (48, T)


def kernel(batch, h0, W_ih0, W_hh0, b_ih0, b_hh0, W_ih, W_hh, b_ih, b_hh,
           Wa, ba, va, fc1_w, fc1_b, fc2_w, fc2_b, _U=128):
    batch = np.asarray(batch, np.float32)
    T = batch.shape[0]
    args = [np.asarray(a, np.float32) for a in
            (h0, W_ih0, W_hh0, b_ih0, b_hh0, W_ih, W_hh, b_ih, b_hh, Wa, ba, va)]
    consts = _build_consts(*args)
    gi0t = _phase_a(batch, args[1], args[3])
    h5 = _run_phase_b(gi0t, consts, T, _U)  # (16, T)
    w_eff = (np.asarray(fc2_w, np.float32) @ np.asarray(fc1_w, np.float32))  # (1,16)
    b_eff = np.asarray(fc2_w, np.float32) @ np.asarray(fc1_b, np.float32) + np.asarray(fc2_b, np.float32)
    out = h5.T @ w_eff[0] + b_eff[0]  # (T,)
    return out[:, None].astype(np.float32)  # (T, 1)
